# revision 13
# baseline (speedup 1.0000x reference)
"""Fan-beam FBP on 8 TRN2 cores — windowed-gather, 4-fold view symmetry.

Views v, v+180, v+360, v+540 share the exact detector-index pattern at
90-degree-rotated base pixels (t,s fan-geometry invariants), so one ap_gather
index stream serves 4 views: the window table T holds the 4 filtered views
interleaved by channel (T[ch] = Q_{ch%4}), each window fetch lands in 16
channels = 4 copies per variant, and a per-variant flat-AP DMA repack extracts
each variant's rows.  The window table is built as 9 shifted contiguous copies
of Q on the scalar engine (no strided writes).  DVE extracts taps with one
stretched-AP multiply against a host-packed per-pixel weight field (shared by
all 4 variants) + segmented reduce.  Host combines the 4 variant accumulators
with rot90s.
"""
import functools
import os
import numpy as np

V, U = 720, 736
NX = NY = 512
SVOX = 400.0
DU = 1.2858
DSO, DOD = 595.0, 490.6
DSD = DSO + DOD
DBETA = 2.0 * np.pi / V

N_CORES = 8
QUADS = 23            # base-view quads per core (cores 4-7: 22 real + dummy)
NVP = 96              # filtered view rows per core (4*23=92, padded)
XBLK = 4              # x blocks of 128 rows
G = 8                 # pixels per gathered window
DW = 18               # window width (fp16 elems; DW*2 bytes % 4 == 0)
NSH = 9               # shift tables (contiguous-copy build of the window table)
ENT = 42              # entries per shift table (e = m // 9, padded)
NELEM = NSH * ENT     # window-base table entries ((m%9)*ENT + m//9 indexing)
HALF = 2              # window-dim halves per block (SBUF budget)
NW = 512 // G         # windows per row (64)
NWH = NW // HALF      # windows per row per half (32)
NIDXW = 16 * NWH      # gather indices per group per instruction (512)
UPAD = 768

_last_exec_ns = None


def _ramp_h():
    n = np.arange(-(U - 1), U)
    h = np.zeros(2 * U - 1, np.float64)
    h[U - 1] = 1.0 / (4.0 * DU * DU)
    odd = (np.abs(n) % 2 == 1)
    h[odd] = -1.0 / (np.pi * n[odd] * DU) ** 2
    return h


def _core_quads(c):
    counts = [23, 23, 23, 23, 22, 22, 22, 22]
    starts = np.cumsum([0] + counts[:-1])
    qs = list(range(starts[c], starts[c] + counts[c]))
    while len(qs) < QUADS:
        qs.append(-1)  # dummy (weights zeroed)
    return qs


@functools.lru_cache(maxsize=1)
def _host_tables():
    h = _ramp_h()
    k = np.arange(U)
    j = np.arange(U)
    Hm = h[(U - 1) + j[None, :] - k[:, None]]
    us = (k - (U - 1) / 2.0) * DU
    cosw = DSD / np.sqrt(DSD * DSD + us * us)
    Hm = (cosw[:, None] * Hm) * DU * (0.5 * DBETA)
    Hp = np.zeros((UPAD, U), np.float32)
    Hp[:U] = Hm.astype(np.float32)

    f32 = np.float32
    dx = f32(SVOX / NX)
    xs = (np.arange(NX, dtype=f32) - f32((NX - 1) / 2.0)) * dx
    X = xs[:, None]
    Y = xs[None, :]
    betas = np.linspace(0.0, 2.0 * np.pi, V, endpoint=False).astype(f32)

    idx_packed = np.empty((N_CORES, QUADS, XBLK, HALF, 128, NIDXW // 16),
                          np.int16)
    w_packed = np.empty((N_CORES, QUADS, XBLK, HALF, 128, NWH * G * DW),
                        np.float16)

    for c in range(N_CORES):
        for qi, vb in enumerate(_core_quads(c)):
            dummy = vb < 0
            vb_eff = 0 if dummy else vb
            cb = np.cos(betas[vb_eff], dtype=f32)
            sb = np.sin(betas[vb_eff], dtype=f32)
            t = X * cb + Y * sb
            s = -X * sb + Y * cb
            D = f32(DSO) - s
            u = f32(DSD) * t / D
            idxf = u / f32(DU) + f32((U - 1) / 2.0)
            mask = (idxf >= 0.0) & (idxf <= U - 1.0)
            i0 = np.clip(np.floor(idxf), 0, U - 2)
            f = idxf - i0.astype(f32)
            w = (f32(DSO) / D) ** 2
            aw = (w * (1.0 - f) * mask).astype(np.float16)
            bw = (w * f * mask).astype(np.float16)
            if dummy:
                aw[:] = 0
                bw[:] = 0
            i0 = i0.astype(np.int32)

            quad = i0.reshape(NX, NW, G)
            base = quad.min(axis=2)
            base -= base & 1                              # even-align
            lam = (quad - base[:, :, None]).astype(np.int64)

            Wq = np.zeros((NX, NW, G, DW), np.float16)
            np.put_along_axis(Wq, lam[..., None], aw.reshape(NX, NW, G, 1),
                              axis=3)
            np.put_along_axis(Wq, lam[..., None] + 1, bw.reshape(NX, NW, G, 1),
                              axis=3)
            Wblk = Wq.reshape(XBLK, 128, HALF, NWH * G * DW)
            w_packed[c, qi] = Wblk.transpose(0, 2, 1, 3)

            m = base >> 1
            ment = (m % 9) * ENT + m // 9  # shift-table entry remap
            Bq = ment.astype(np.int16).reshape(XBLK, 8, 16, HALF, NWH // 16, 16)
            idx_packed[c, qi] = Bq.transpose(0, 3, 1, 5, 2, 4).reshape(
                XBLK, HALF, 128, NIDXW // 16)

    return Hp, idx_packed, w_packed


@functools.lru_cache(maxsize=1)
def _build_module():
    import concourse.bacc as bacc
    import concourse.mybir as mybir
    import concourse.tile as tile
    from concourse import library_config

    f32 = mybir.dt.float32
    f16 = mybir.dt.float16
    i16 = mybir.dt.int16

    nc = bacc.Bacc("TRN2", target_bir_lowering=False, debug=False,
                   num_devices=N_CORES)
    sinoT_d = nc.dram_tensor("sinot", [UPAD, NVP], f32, kind="ExternalInput")
    hmat_d = nc.dram_tensor("hmat", [UPAD, U], f32, kind="ExternalInput")
    sel_d = nc.dram_tensor("sel", [4, 128], f16, kind="ExternalInput")
    idx_d = nc.dram_tensor("idxs", [QUADS, XBLK, HALF, 128, NIDXW // 16], i16,
                           kind="ExternalInput")
    w_d = nc.dram_tensor("wq", [QUADS, XBLK, HALF, 128, NWH * G * DW], f16,
                         kind="ExternalInput")
    out_d = nc.dram_tensor("out", [4, NX, NY], f32, kind="ExternalOutput")

    with tile.TileContext(nc) as tc:
        nc.gpsimd.load_library(library_config.ap_gather)
        with (
            tc.tile_pool(name="const", bufs=1) as constp,
            tc.tile_pool(name="psum", bufs=2, space="PSUM") as psump,
            tc.tile_pool(name="tab", bufs=2) as tabp,
            tc.tile_pool(name="bcast", bufs=2) as bcp,
            tc.tile_pool(name="ot", bufs=3) as otp,
            tc.tile_pool(name="g", bufs=6) as gp,
            tc.tile_pool(name="stream", bufs=3) as strp,
            tc.tile_pool(name="m", bufs=3) as mp,
            tc.tile_pool(name="acc", bufs=1) as accp,
        ):
            # ---- load filter inputs ----
            sin_sb = constp.tile([128, 6 * NVP], f32)
            h_sb = constp.tile([128, 6 * U], f32)
            for i in range(6):
                nc.sync.dma_start(
                    sin_sb[:, i * NVP:(i + 1) * NVP],
                    sinoT_d.ap()[i * 128:(i + 1) * 128, :])
                nc.sync.dma_start(
                    h_sb[:, i * U:(i + 1) * U],
                    hmat_d.ap()[i * 128:(i + 1) * 128, :])

            # ---- ramp filter ----
            qf = constp.tile([NVP, UPAD], f16)
            nc.vector.memset(qf[:], 0.0)
            for jc in range(2):
                ps = psump.tile([NVP, U // 2], f32)
                for kt in range(6):
                    nc.tensor.matmul(
                        ps[:],
                        sin_sb[:, kt * NVP:(kt + 1) * NVP],
                        h_sb[:, kt * U + jc * (U // 2): kt * U + (jc + 1) * (U // 2)],
                        start=(kt == 0), stop=(kt == 5))
                nc.vector.tensor_copy(qf[:, jc * (U // 2):(jc + 1) * (U // 2)],
                                      ps[:])

            # ---- accumulators: 4 variants x 4 xblocks ----
            accs = [[None] * XBLK for _ in range(4)]
            for kv in range(4):
                for b in range(XBLK):
                    acc_t = accp.tile([128, 512], f32, tag=f"acc{kv}{b}")
                    accs[kv][b] = acc_t
                    nc.vector.memset(acc_t[:], 0.0)

            sel = constp.tile([4, 128], f16)
            nc.sync.dma_start(sel[:], sel_d.ap())

            # ---- main loop over view quads (table build pipelined) ----
            def build_T(a):
                Sn = bcp.tile([4, UPAD], f16, tag="Srows")
                for kv in range(4):
                    row = 4 * a + kv
                    nc.scalar.dma_start(Sn[kv:kv + 1, :], qf[row:row + 1, :])
                bps = psump.tile([128, UPAD], f32, tag="bpsum")
                for n0, n1 in ((0, 512), (512, UPAD)):
                    nc.tensor.matmul(bps[:, n0:n1], sel[:], Sn[:, n0:n1],
                                     start=True, stop=True)
                # window table as 9 shift tables: T[(m%9)*ENT + m//9] =
                # Q[2m : 2m+DW]  ->  T_s[e*DW + w] = Q[18e + 2s + w],
                # a contiguous shifted copy of Q per s (runs on Act engine)
                T = tabp.tile([128, DW * NELEM], f16)
                for s in range(NSH):
                    ent_s = (367 - s) // NSH + 1
                    nc.scalar.copy(
                        T[:, s * ENT * DW: s * ENT * DW + ent_s * DW],
                        bps[:, 2 * s: 2 * s + ent_s * DW])
                return T

            T_cur = build_T(0)
            for a in range(QUADS):
                T_next = build_T(a + 1) if a + 1 < QUADS else None
                T = T_cur

                for b in range(XBLK):
                    for hh in range(HALF):
                        it = strp.tile([128, NIDXW // 16], i16, tag="idx")
                        nc.sync.dma_start(it[:], idx_d.ap()[a, b, hh])
                        wt = strp.tile([128, NWH * G * DW], f16, tag="wq")
                        nc.sync.dma_start(wt[:], w_d.ap()[a, b, hh])

                        ot = otp.tile([128, NIDXW * DW], f16)
                        nc.gpsimd.ap_gather(ot[:], T[:], it[:],
                                            channels=128, num_elems=NELEM, d=DW,
                                            num_idxs=NIDXW)

                        for kv in range(4):
                            ga = gp.tile([128, NWH * DW], f16, tag="g")
                            row_len = NIDXW * DW
                            src = ot[:].copy()
                            src.ap = type(src.ap)(
                                [[16 * row_len, 8], [NWH * DW, 16], [1, NWH * DW]])
                            src.offset = kv * row_len
                            nc.scalar.dma_start(ga[:], src)

                            # stretched read: element (jw, i, w) = ga[jw*DW + w]
                            gread = ga[:].copy()
                            gread.ap = type(gread.ap)(
                                [[NWH * DW, 128], [DW, NWH], [0, G], [1, DW]])
                            m = mp.tile([128, NWH * G * DW], f16, tag="m")
                            nc.vector.tensor_mul(m[:], gread, wt[:])
                            sj = mp.tile([128, NWH * G], f32, tag="s")
                            mr = m[:].copy()
                            mr.ap = type(mr.ap)(
                                [[NWH * G * DW, 128], [DW, NWH * G], [1, DW]])
                            nc.vector.reduce_sum(sj[:], mr,
                                                 axis=mybir.AxisListType.X)
                            y0 = hh * (NWH * G)
                            nc.vector.tensor_add(
                                accs[kv][b][:, y0:y0 + NWH * G],
                                accs[kv][b][:, y0:y0 + NWH * G], sj[:])
                T_cur = T_next

            for kv in range(4):
                for b in range(XBLK):
                    nc.sync.dma_start(out_d.ap()[kv, b * 128:(b + 1) * 128, :],
                                      accs[kv][b][:])

    nc.compile()
    return nc


def kernel(sinogram: np.ndarray) -> np.ndarray:
    global _last_exec_ns
    from concourse import bass_utils

    Hp, idx_packed, w_packed = _host_tables()
    nc = _build_module()

    sino = np.asarray(sinogram, np.float32).reshape(V, U)
    sel = np.zeros((4, 128), np.float16)
    for kv in range(4):
        sel[kv, kv::4] = 1.0

    in_maps = []
    for c in range(N_CORES):
        vs = []
        for vb in _core_quads(c):
            vb_eff = 0 if vb < 0 else vb
            vs.extend([vb_eff + 180 * kk for kk in range(4)])
        st = np.zeros((UPAD, NVP), np.float32)
        st[:U, :len(vs)] = sino[vs, :].T
        in_maps.append({
            "sinot": st,
            "hmat": Hp,
            "sel": sel,
            "idxs": idx_packed[c],
            "wq": w_packed[c],
        })

    trace = bool(int(os.environ.get("FBP_TRACE", "0")))
    kw = {}
    if trace:
        tdir = os.environ.get("FBP_TRACE_DIR")
        if not tdir:
            import tempfile
            tdir = tempfile.mkdtemp()
        os.makedirs(tdir, exist_ok=True)
        kw = dict(trace=True, tmpdir=tdir)
    res = bass_utils.run_bass_kernel_spmd(nc, in_maps,
                                          core_ids=list(range(N_CORES)), **kw)
    _last_exec_ns = res.exec_time_ns

    acc = [np.zeros((NX, NY), np.float64) for _ in range(4)]
    for c in range(N_CORES):
        o = res.results[c]["out"]
        for kv in range(4):
            acc[kv] += o[kv]
    img = np.zeros((NX, NY), np.float64)
    for kv in range(4):
        img += np.rot90(acc[kv], kv)
    return img.astype(np.float32).reshape(1, 1, NX, NY)
